# revision 18
# baseline (speedup 1.0000x reference)
"""Trainium2 Bass kernel for the AttentionModule problem.

Cross-attention with normalized-position RoPE:
  q = Wq @ x;  k = Wk @ ctx;  v = Wv @ ctx  (per-head RoPE on q, k)
  out = Wo @ (softmax(q^T k / sqrt(512)) @ v), masked.

Sharding: 8 cores = 4 batches x 2 T-halves. Each core computes the full
module for (batch b, query half th) with all heads; host concatenates.
No collectives needed.

v2 design notes (vs the phase-serial v1):
- All inputs (x, ctx, weights, rope tables) are cast/laid out on the host
  and DMA'd directly as bf16 -- no on-device cast pass, no on-device
  sin/cos generation.
- The softmax exp stream on the Scalar engine (~142us of [128,1024] Exp
  ops) is the intrinsic bottleneck, so the kernel is organized to keep
  ACT busy continuously from ~15us onward: only Q/K chunk 0 (plus a bit
  of chunk 1) is projected up front, and the remaining projection groups
  (V, Q/K chunks 1-3, output projection) are drip-fed one group per
  attention inner iteration into the Tensor engine's spare cycles.
- S matmuls are emitted at high priority (tc.high_priority) so they
  preempt queued O/projection matmuls; the two heads of a pair use
  row-disjoint 64-partition lhsT (auto tile_position 0/64) and are
  emitted adjacently for PE row-group dual-issue.
- Softmax normalization: the per-head denominator row (row 64 of the O
  psum, via the ones-column of V1) is DMA'd into a shared [8,T] tile,
  inverted with one reciprocal_approx_fast, broadcast with gpsimd, and
  multiplied straight out of PSUM (the v1 per-head [1,T] DVE reciprocal
  cost 52us on one lane).
- PSUM budget (8 banks): 2 x s-slots [128,1024] (4) shared round-robin
  between S tiles and projection-group psums, + po_a/po_b [65,1024] (4).

Layouts on device (feature-major, partition = feature):
  x    [128, dc, T]   ctx [128, dc, L]      (host-packed bf16)
  Q/K  [128, ac, T|L] via twin projection with host-rotated weights
       (Qr = (R'Wq) @ x) and host-computed bf16 sin/cos tables.
  S_h  [l, t] = K_h^T Q_h (row-paired head pairs on the PE array)
  E    = exp(S/scale + log cmask)  (ACT; no max-subtraction: |logits|<~0.5)
  O_h  [65, t] = [V_h | 1]^T E  -> row 64 holds softmax denominators
  out  [dm, t] = Wo^T (O / s) * xmask   (DMA'd straight from PSUM when
       xmask is all-ones)
"""

import math
import sys
import types

sys.path.insert(0, "/opt/trn_rl_repo")

import numpy as np
import ml_dtypes

import concourse.bass as bass
import concourse.tile as tile
from concourse import bacc, mybir
from concourse.bass_utils import run_bass_kernel_spmd

# Problem constants (hardcoded per spec; kernel.py must be self-contained)
D_MODEL = 512
D_CONTEXT = 512
NUM_HEADS = 8
ATTN_DIM = 512
HEAD_DIM = 64
ROPE_GAMMA = 10.0
ATTN_SCALE = math.sqrt(ATTN_DIM)
B = 4
T_FULL = 2048
L = 2048
N_CORES = 8
T = T_FULL // 2  # per-core query slice
P = 128
NAC = ATTN_DIM // P  # 4 chunks of 128 on the feature dim
NLC = L // P  # 16 l-chunks
FP32 = mybir.dt.float32
BF16 = mybir.dt.bfloat16
AF = mybir.ActivationFunctionType
ALU = mybir.AluOpType
BF16NP = ml_dtypes.bfloat16

_GRAPH_CACHE = {}


def _ensure_ntff_hook():
    """antenv.axon_hooks is absent in some images; inject it so trace=True
    can produce exec_time_ns. Harmless if tracing is never requested."""
    if "antenv.axon_hooks" in sys.modules:
        return
    try:
        mod = types.ModuleType("antenv.axon_hooks")
        mod._hook = None
        mod.set_axon_ntff_profile_hook = lambda h: setattr(mod, "_hook", h)
        mod.get_axon_ntff_profile_hook = lambda: mod._hook
        sys.modules["antenv.axon_hooks"] = mod
        from trn_agent_boot.trn_boot import _ntff_profile_via_ctypes

        mod.set_axon_ntff_profile_hook(
            _ntff_profile_via_ctypes("/opt/axon/libaxon_pjrt.so")
        )
    except Exception:
        pass


def _build_graph(use_bias: bool, use_cmask: bool, use_xmask: bool, dbg: bool = False):
    nc = bacc.Bacc("TRN2", target_bir_lowering=False, debug=False, num_devices=N_CORES)

    x_d = nc.dram_tensor("x", [P, NAC * T], BF16, kind="ExternalInput").ap()
    ctx_d = nc.dram_tensor("ctxT", [P, NAC * L], BF16, kind="ExternalInput").ap()
    w_d = {
        name: nc.dram_tensor(name, [P, NAC * 512], BF16, kind="ExternalInput").ap()
        for name in ("wq", "wqr", "wk", "wkr", "wv", "wo")
    }
    tabs_d = nc.dram_tensor("tabs", [P, 2 * T + 2 * L], BF16, kind="ExternalInput").ap()
    if use_bias:
        bias_d = nc.dram_tensor("biases", [1, 6 * 512], BF16, kind="ExternalInput").ap()
    if use_cmask:
        logcm_d = nc.dram_tensor("logcm", [P, NLC], FP32, kind="ExternalInput").ap()
    if use_xmask:
        xmask_d = nc.dram_tensor("xmaskb", [P, T], FP32, kind="ExternalInput").ap()
    out_d = nc.dram_tensor("out", [D_MODEL, T], FP32, kind="ExternalOutput").ap()
    if dbg:
        dbg_d = {
            "qrope": nc.dram_tensor("d_qrope", [P, NAC * T], BF16, kind="ExternalOutput").ap(),
            "krope": nc.dram_tensor("d_krope", [P, NAC * L], BF16, kind="ExternalOutput").ap(),
            "v1": nc.dram_tensor("d_v1", [P, NLC * NUM_HEADS * (HEAD_DIM + 1)], BF16, kind="ExternalOutput").ap(),
            "onorm": nc.dram_tensor("d_onorm", [P, NAC * T], BF16, kind="ExternalOutput").ap(),
            "oraw": nc.dram_tensor("d_oraw", [HEAD_DIM + 1, NUM_HEADS * T], FP32, kind="ExternalOutput").ap(),
            "rec": nc.dram_tensor("d_rec", [1, T], FP32, kind="ExternalOutput").ap(),
            "rb": nc.dram_tensor("d_rb", [HEAD_DIM, T], FP32, kind="ExternalOutput").ap(),
        }

    inv_scale = 1.0 / ATTN_SCALE
    NTG = T // 512  # 2 query column groups of 512
    NLG = L // 512  # 4 key column groups of 512

    with tile.TileContext(nc) as tc:
        with (
            tc.tile_pool(name="const", bufs=1) as const,
            tc.tile_pool(name="big", bufs=1) as big,
            tc.tile_pool(name="tmp", bufs=2) as tmp_pool,
            tc.tile_pool(name="rbp", bufs=2) as rb_pool,
            tc.tile_pool(name="outp", bufs=2) as out_pool,
            tc.tile_pool(name="epool", bufs=2) as e_pool,
        ):
            # ---- input tiles (DMA'd directly as bf16, host layout) ----
            w_bf = {}
            for name in ("wq", "wqr", "wk", "wkr", "wv", "wo"):
                w_bf[name] = big.tile(
                    [P, NAC, 512], BF16, tag=f"w_{name}", name=f"w_{name}"
                )
            x_bf = big.tile([P, NAC, T], BF16)
            ctx_bf = big.tile([P, NAC, L], BF16)
            tabs = const.tile([P, 2 * T + 2 * L], BF16)
            cos_q = tabs[:, 0:T]
            sin_q = tabs[:, T : 2 * T]
            cos_k = tabs[:, 2 * T : 2 * T + L]
            sin_k = tabs[:, 2 * T + L : 2 * T + 2 * L]

            # DMA order = need order: Q-projection inputs first, then K, V, O
            nc.sync.dma_start(x_bf[:], x_d.rearrange("p (c t) -> p c t", c=NAC))
            for name in ("wq", "wqr"):
                nc.sync.dma_start(
                    w_bf[name][:], w_d[name].rearrange("p (c a) -> p c a", c=NAC)
                )
            nc.sync.dma_start(tabs[:], tabs_d[:])
            nc.sync.dma_start(ctx_bf[:], ctx_d.rearrange("p (c l) -> p c l", c=NAC))
            for name in ("wk", "wkr", "wv", "wo"):
                nc.sync.dma_start(
                    w_bf[name][:], w_d[name].rearrange("p (c a) -> p c a", c=NAC)
                )

            zero_b = const.tile([P, 1], FP32)
            nc.vector.memset(zero_b[:], 0.0)
            if use_cmask:
                logcm_sb = const.tile([P, NLC], FP32)
                nc.sync.dma_start(logcm_sb[:], logcm_d[:])
            if use_xmask:
                xmask_sb = const.tile([P, T], FP32)
                nc.sync.dma_start(xmask_sb[:], xmask_d[:])
            if use_bias:
                bias_bf = const.tile([1, 6, 512], BF16)
                nc.sync.dma_start(
                    bias_bf[:], bias_d.rearrange("p (b a) -> p b a", b=6)
                )
                ones_row = const.tile([1, 512], BF16)
                nc.vector.memset(ones_row[:], 1.0)
                ones_col = const.tile([1, P], BF16)
                nc.vector.memset(ones_col[:], 1.0)

            q_rope = big.tile([P, NAC, T], BF16)
            k_rope = big.tile([P, NAC, L], BF16)
            v1 = big.tile([P, NLC, NUM_HEADS, HEAD_DIM + 1], BF16)
            nc.vector.memset(v1[:, :, :, HEAD_DIM : HEAD_DIM + 1], 1.0)
            o_norm = big.tile([P, NAC, T], BF16)

            with (
                tc.tile_pool(name="psS", bufs=2, space="PSUM") as psS,
                tc.tile_pool(name="psO", bufs=2, space="PSUM") as psO,
            ):
                # ---- projection groups: each uses one [128,1024] s-slot ----
                def rope_combine(out_ap, ps, cos_ap, sin_ap):
                    tt = tmp_pool.tile([P, 512], BF16, tag="ropetmp")
                    nc.vector.tensor_tensor(tt[:], ps[:, 512:1024], sin_ap, op=ALU.mult)
                    nc.vector.tensor_tensor(out_ap, ps[:, 0:512], cos_ap, op=ALU.mult)
                    nc.vector.tensor_tensor(out_ap, out_ap, tt[:], op=ALU.add)

                def q_group(ac, tg):
                    sl = slice(tg * 512, (tg + 1) * 512)
                    ps = psS.tile([P, 1024], FP32, tag="s")
                    for half, wn, bb in ((0, "wq", 0), (1, "wqr", 1)):
                        psl = slice(half * 512, half * 512 + 512)
                        for dc in range(NAC):
                            nc.tensor.matmul(
                                ps[:, psl],
                                lhsT=w_bf[wn][:, dc, ac * P : (ac + 1) * P],
                                rhs=x_bf[:, dc, sl],
                                start=(dc == 0),
                                stop=(dc == NAC - 1) and not use_bias,
                            )
                        if use_bias:
                            nc.tensor.matmul(
                                ps[:, psl],
                                lhsT=bias_bf[:, bb, ac * P : (ac + 1) * P],
                                rhs=ones_row[:],
                                start=False,
                                stop=True,
                            )
                    rope_combine(q_rope[:, ac, sl], ps, cos_q[:, sl], sin_q[:, sl])

                def k_group(ac, g):
                    sl = slice(g * 512, (g + 1) * 512)
                    ps = psS.tile([P, 1024], FP32, tag="s")
                    for half, wn, bb in ((0, "wk", 2), (1, "wkr", 3)):
                        psl = slice(half * 512, half * 512 + 512)
                        for dc in range(NAC):
                            nc.tensor.matmul(
                                ps[:, psl],
                                lhsT=w_bf[wn][:, dc, ac * P : (ac + 1) * P],
                                rhs=ctx_bf[:, dc, sl],
                                start=(dc == 0),
                                stop=(dc == NAC - 1) and not use_bias,
                            )
                        if use_bias:
                            nc.tensor.matmul(
                                ps[:, psl],
                                lhsT=bias_bf[:, bb, ac * P : (ac + 1) * P],
                                rhs=ones_row[:],
                                start=False,
                                stop=True,
                            )
                    rope_combine(k_rope[:, ac, sl], ps, cos_k[:, sl], sin_k[:, sl])

                def v_group(g):
                    # V^T for l-chunks 2g, 2g+1 -> v1[:, lc, h, 0:64]
                    ps = psS.tile([P, 1024], FP32, tag="s")
                    for i in range(2):
                        lc = 2 * g + i
                        psl = slice(i * 512, i * 512 + 512)
                        for dc in range(NAC):
                            nc.tensor.matmul(
                                ps[:, psl],
                                lhsT=ctx_bf[:, dc, lc * P : (lc + 1) * P],
                                rhs=w_bf["wv"][:, dc, :],
                                start=(dc == 0),
                                stop=(dc == NAC - 1) and not use_bias,
                            )
                        if use_bias:
                            nc.tensor.matmul(
                                ps[:, psl],
                                lhsT=ones_col[:],
                                rhs=bias_bf[:, 4, :],
                                start=False,
                                stop=True,
                            )
                        nc.vector.tensor_copy(
                            v1[:, lc, :, 0:HEAD_DIM],
                            ps[:, psl].rearrange("p (h d) -> p h d", d=HEAD_DIM),
                        )

                def out_group(dmc):
                    ps = psS.tile([P, 1024], FP32, tag="s")
                    for tg in range(NTG):
                        sl = slice(tg * 512, (tg + 1) * 512)
                        for ac in range(NAC):
                            nc.tensor.matmul(
                                ps[:, sl],
                                lhsT=w_bf["wo"][:, ac, dmc * P : (dmc + 1) * P],
                                rhs=o_norm[:, ac, sl],
                                start=(ac == 0),
                                stop=(ac == NAC - 1) and not use_bias,
                            )
                        if use_bias:
                            nc.tensor.matmul(
                                ps[:, sl],
                                lhsT=bias_bf[:, 5, dmc * P : (dmc + 1) * P],
                                rhs=ones_row[:],
                                start=False,
                                stop=True,
                            )
                    ot = out_pool.tile([P, T], FP32, tag="ot")
                    if use_xmask:
                        nc.vector.tensor_tensor(ot[:], ps[:], xmask_sb[:], op=ALU.mult)
                    else:
                        nc.vector.tensor_copy(ot[:], ps[:])
                    nc.sync.dma_start(out_d[dmc * P : (dmc + 1) * P, :], ot[:])

                # ---- upfront projections: enough to start + sustain hp0 ----
                for tg in range(NTG):
                    q_group(0, tg)
                for g in range(NLG):
                    k_group(0, g)
                for tg in range(NTG):
                    q_group(1, tg)
                k_group(1, 0)

                # drip-feed queue: one group per attention inner iteration.
                # V group g must land before O(hp0, q4=g) (emitted at iter g+1).
                drip = [[], [], [], []]
                drip[0] = [lambda g=g: v_group(g) for g in range(8)]
                drip[1] = (
                    [lambda g=g: k_group(1, g) for g in (1, 2, 3)]
                    + [lambda tg=tg: q_group(2, tg) for tg in range(NTG)]
                    + [lambda: k_group(2, 0)]
                )
                drip[2] = (
                    [lambda g=g: k_group(2, g) for g in (1, 2, 3)]
                    + [lambda tg=tg: q_group(3, tg) for tg in range(NTG)]
                    + [lambda: k_group(3, 0)]
                )
                drip[3] = [lambda g=g: k_group(3, g) for g in (1, 2, 3)]

                # ---- attention, one head pair at a time ----
                for hp in range(NAC):
                    h_a, h_b = 2 * hp, 2 * hp + 1
                    queue = list(drip[hp])
                    po_a = psO.tile([HEAD_DIM + 1, T], FP32, tag="po")
                    po_b = psO.tile([HEAD_DIM + 1, T], FP32, tag="po")

                    def emit_o(q4, e_a, e_b, po_a=po_a, po_b=po_b, h_a=h_a, h_b=h_b):
                        for tg in range(NTG):
                            sl = slice(tg * 512, (tg + 1) * 512)
                            for lc4 in range(2):
                                lc = q4 * 2 + lc4
                                nc.tensor.matmul(
                                    po_a[:, sl],
                                    lhsT=v1[:, lc, h_a, :],
                                    rhs=e_a[:, lc4, sl],
                                    start=(lc == 0),
                                    stop=(lc == NLC - 1),
                                )
                                nc.tensor.matmul(
                                    po_b[:, sl],
                                    lhsT=v1[:, lc, h_b, :],
                                    rhs=e_b[:, lc4, sl],
                                    start=(lc == 0),
                                    stop=(lc == NLC - 1),
                                )

                    pending = None
                    for q4 in range(8):
                        e_a = e_pool.tile([P, 2, T], BF16, tag="eA")
                        e_b = e_pool.tile([P, 2, T], BF16, tag="eB")
                        for lc4 in range(2):
                            lc = q4 * 2 + lc4
                            s_a = psS.tile([P, T], FP32, tag="s")
                            s_b = psS.tile([P, T], FP32, tag="s")
                            with tc.high_priority():
                                for tg in range(NTG):
                                    sl = slice(tg * 512, (tg + 1) * 512)
                                    nc.tensor.matmul(
                                        s_a[:, sl],
                                        lhsT=k_rope[0:64, hp, lc * P : (lc + 1) * P],
                                        rhs=q_rope[0:64, hp, sl],
                                        start=True,
                                        stop=True,
                                    )
                                    nc.tensor.matmul(
                                        s_b[:, sl],
                                        lhsT=k_rope[64:128, hp, lc * P : (lc + 1) * P],
                                        rhs=q_rope[64:128, hp, sl],
                                        start=True,
                                        stop=True,
                                    )
                            eb = logcm_sb[:, lc : lc + 1] if use_cmask else zero_b[:]
                            nc.scalar.activation(
                                e_a[:, lc4], s_a[:], AF.Exp, bias=eb, scale=inv_scale
                            )
                            nc.scalar.activation(
                                e_b[:, lc4], s_b[:], AF.Exp, bias=eb, scale=inv_scale
                            )
                        if pending is not None:
                            emit_o(*pending)
                        if queue:
                            queue.pop(0)()
                        pending = (q4, e_a, e_b)
                    emit_o(*pending)

                    # ---- normalization for this head pair ----
                    # drain po to SBUF fp32 (frees the psum slot), invert the
                    # denominator row on one DVE lane, gpsimd-broadcast it
                    # across 64 partitions, multiply.
                    for h, po in ((h_a, po_a), (h_b, po_b)):
                        o_raw = rb_pool.tile([HEAD_DIM + 1, T], FP32, tag="oraw")
                        nc.vector.tensor_copy(o_raw[:], po[:])
                        if dbg:
                            nc.sync.dma_start(
                                dbg_d["oraw"][:, h * T : (h + 1) * T], o_raw[:]
                            )
                        onorm_ap = o_norm[(h % 2) * 64 : (h % 2) * 64 + 64, hp, :]
                        rec = rb_pool.tile([1, T], FP32, tag="rec")
                        nc.vector.reciprocal(rec[:], o_raw[64:65, :])
                        rb = rb_pool.tile([HEAD_DIM, T], FP32, tag="rb")
                        nc.gpsimd.partition_broadcast(
                            rb[:], rec[:], channels=HEAD_DIM
                        )
                        nc.vector.tensor_tensor(
                            onorm_ap, o_raw[0:HEAD_DIM, :], rb[:], op=ALU.mult
                        )
                        if dbg and h == 0:
                            nc.sync.dma_start(dbg_d["rec"][:], rec[:])
                            nc.sync.dma_start(dbg_d["rb"][:], rb[:])

                # ---- output projection + DMA out ----
                for dmc in range(NAC):
                    out_group(dmc)

                if dbg:
                    nc.sync.dma_start(
                        dbg_d["qrope"][:], q_rope[:].rearrange("p a b -> p (a b)")
                    )
                    nc.sync.dma_start(
                        dbg_d["krope"][:], k_rope[:].rearrange("p a b -> p (a b)")
                    )
                    nc.sync.dma_start(
                        dbg_d["v1"][:], v1[:].rearrange("p a b c -> p (a b c)")
                    )
                    nc.sync.dma_start(
                        dbg_d["onorm"][:], o_norm[:].rearrange("p a b -> p (a b)")
                    )

    nc.compile()
    return nc


def _rot_rows(w: np.ndarray) -> np.ndarray:
    """Apply the rotate-half permutation R' on the attn-dim axis (rows):
    row (h,j<32) <- -row (h,32+j);  row (h,32+j) <- +row (h,j)."""
    out = np.empty_like(w)
    for h in range(NUM_HEADS):
        blk = w[h * HEAD_DIM : (h + 1) * HEAD_DIM]
        out[h * HEAD_DIM : h * HEAD_DIM + 32] = -blk[32:64]
        out[h * HEAD_DIM + 32 : (h + 1) * HEAD_DIM] = blk[0:32]
    return out


def _featmajor(w: np.ndarray) -> np.ndarray:
    """[512, N] -> [128, 4, N] bf16 with rows chunked (c*128+p -> [p, c])."""
    n = w.shape[1]
    return np.ascontiguousarray(
        w.reshape(NAC, P, n).transpose(1, 0, 2).reshape(P, NAC * n).astype(BF16NP)
    )


def kernel(
    x,
    context,
    x_mask,
    context_mask,
    Wq_w,
    Wq_b,
    Wk_w,
    Wk_b,
    Wv_w,
    Wv_b,
    Wo_w,
    Wo_b,
    _want_trace=False,
):
    _ensure_ntff_hook()
    x = np.asarray(x, np.float32)
    context = np.asarray(context, np.float32)
    x_mask = np.asarray(x_mask, np.float32)
    context_mask = np.asarray(context_mask, np.float32)
    weights = {
        "wq": _featmajor(np.asarray(Wq_w, np.float32).T),
        "wqr": _featmajor(_rot_rows(np.asarray(Wq_w, np.float32)).T),
        "wk": _featmajor(np.asarray(Wk_w, np.float32).T),
        "wkr": _featmajor(_rot_rows(np.asarray(Wk_w, np.float32)).T),
        "wv": _featmajor(np.asarray(Wv_w, np.float32).T),
        "wo": _featmajor(np.asarray(Wo_w, np.float32).T),
    }
    biases = np.stack(
        [
            np.asarray(Wq_b, np.float32),
            _rot_rows(np.asarray(Wq_b, np.float32)[:, None])[:, 0],
            np.asarray(Wk_b, np.float32),
            _rot_rows(np.asarray(Wk_b, np.float32)[:, None])[:, 0],
            np.asarray(Wv_b, np.float32),
            np.asarray(Wo_b, np.float32),
        ]
    )  # [6, 512]

    use_bias = bool(np.any(biases != 0.0))
    use_cmask = not bool(np.all(context_mask == 1.0))
    use_xmask = not bool(np.all(x_mask == 1.0))

    key = (use_bias, use_cmask, use_xmask)
    if key not in _GRAPH_CACHE:
        _GRAPH_CACHE[key] = _build_graph(*key)
    nc = _GRAPH_CACHE[key]

    len_q = x_mask.sum(axis=(1, 2))  # [B]
    len_k = context_mask.sum(axis=(1, 2))
    theta = (1.0 / (10000.0 ** (np.arange(32, dtype=np.float64) / 32.0))) * ROPE_GAMMA
    theta128 = np.tile(theta, 4)[:, None]  # [128, 1]; row p -> theta_{p%32}

    in_maps = []
    for c in range(N_CORES):
        b, th = c // 2, c % 2
        t0 = th * T
        ang_q = theta128 * ((t0 + np.arange(T)) / len_q[b])[None, :]
        ang_k = theta128 * (np.arange(L) / len_k[b])[None, :]
        tabs = np.concatenate(
            [np.cos(ang_q), np.sin(ang_q), np.cos(ang_k), np.sin(ang_k)], axis=1
        ).astype(BF16NP)
        m = {
            "x": _featmajor(x[b, :, t0 : t0 + T]),
            "ctxT": _featmajor(np.ascontiguousarray(context[b].T)),
            "tabs": np.ascontiguousarray(tabs),
            **weights,
        }
        if use_bias:
            m["biases"] = np.ascontiguousarray(biases.reshape(1, -1).astype(BF16NP))
        if use_cmask:
            with np.errstate(divide="ignore"):
                lcm = np.log(context_mask[b, 0]).astype(np.float32)  # [L]
            m["logcm"] = np.ascontiguousarray(lcm.reshape(NLC, P).T)
        if use_xmask:
            m["xmaskb"] = np.ascontiguousarray(
                np.broadcast_to(x_mask[b, 0, t0 : t0 + T], (P, T))
            )
        in_maps.append(m)

    res = run_bass_kernel_spmd(
        nc, in_maps, core_ids=list(range(N_CORES)), trace=_want_trace
    )
    out = np.empty((B, D_MODEL, T_FULL), np.float32)
    for c in range(N_CORES):
        b, th = c // 2, c % 2
        out[b, :, th * T : (th + 1) * T] = res.results[c]["out"]
    if _want_trace:
        return out, res
    return out


# revision 26
# speedup vs baseline: 1.2174x; 1.2174x over previous
"""Trainium2 Bass kernel for the AttentionModule problem.

Cross-attention with normalized-position RoPE:
  q = Wq @ x;  k = Wk @ ctx;  v = Wv @ ctx  (per-head RoPE on q, k)
  out = Wo @ (softmax(q^T k / sqrt(512)) @ v), masked.

Sharding: 8 cores = 4 batches x 2 T-halves. Each core computes the full
module for (batch b, query half th) with all heads; host concatenates.
No collectives needed.

v2 design notes (vs the phase-serial v1):
- All inputs (x, ctx, weights, rope tables) are cast/laid out on the host
  and DMA'd directly as bf16 -- no on-device cast pass, no on-device
  sin/cos generation.
- The softmax exp stream on the Scalar engine (~142us of [128,1024] Exp
  ops) is the intrinsic bottleneck, so the kernel is organized to keep
  ACT busy continuously from ~15us onward: only Q/K chunk 0 (plus a bit
  of chunk 1) is projected up front, and the remaining projection groups
  (V, Q/K chunks 1-3, output projection) are drip-fed one group per
  attention inner iteration into the Tensor engine's spare cycles.
- S matmuls are emitted at high priority (tc.high_priority) so they
  preempt queued O/projection matmuls; the two heads of a pair use
  row-disjoint 64-partition lhsT (auto tile_position 0/64) and are
  emitted adjacently for PE row-group dual-issue.
- Softmax normalization: the per-head denominator row (row 64 of the O
  psum, via the ones-column of V1) is DMA'd into a shared [8,T] tile,
  inverted with one reciprocal_approx_fast, broadcast with gpsimd, and
  multiplied straight out of PSUM (the v1 per-head [1,T] DVE reciprocal
  cost 52us on one lane).
- PSUM budget (8 banks): 2 x s-slots [128,1024] (4) shared round-robin
  between S tiles and projection-group psums, + po_a/po_b [65,1024] (4).

Layouts on device (feature-major, partition = feature):
  x    [128, dc, T]   ctx [128, dc, L]      (host-packed bf16)
  Q/K  [128, ac, T|L] via twin projection with host-rotated weights
       (Qr = (R'Wq) @ x) and host-computed bf16 sin/cos tables.
  S_h  [l, t] = K_h^T Q_h (row-paired head pairs on the PE array)
  E    = exp(S/scale + log cmask)  (ACT; no max-subtraction: |logits|<~0.5)
  O_h  [65, t] = [V_h | 1]^T E  -> row 64 holds softmax denominators
  out  [dm, t] = Wo^T (O / s) * xmask   (DMA'd straight from PSUM when
       xmask is all-ones)
"""

import math
import sys
import types

sys.path.insert(0, "/opt/trn_rl_repo")

import numpy as np
import ml_dtypes

import concourse.bass as bass
import concourse.tile as tile
from concourse import bacc, mybir
from concourse.bass_utils import run_bass_kernel_spmd

# Problem constants (hardcoded per spec; kernel.py must be self-contained)
D_MODEL = 512
D_CONTEXT = 512
NUM_HEADS = 8
ATTN_DIM = 512
HEAD_DIM = 64
ROPE_GAMMA = 10.0
ATTN_SCALE = math.sqrt(ATTN_DIM)
B = 4
T_FULL = 2048
L = 2048
N_CORES = 8
T = T_FULL // 2  # per-core query slice
P = 128
NAC = ATTN_DIM // P  # 4 chunks of 128 on the feature dim
NLC = L // P  # 16 l-chunks
FP32 = mybir.dt.float32
BF16 = mybir.dt.bfloat16
AF = mybir.ActivationFunctionType
ALU = mybir.AluOpType
BF16NP = ml_dtypes.bfloat16

_GRAPH_CACHE = {}


def _ensure_ntff_hook():
    """antenv.axon_hooks is absent in some images; inject it so trace=True
    can produce exec_time_ns. Harmless if tracing is never requested."""
    if "antenv.axon_hooks" in sys.modules:
        return
    try:
        mod = types.ModuleType("antenv.axon_hooks")
        mod._hook = None
        mod.set_axon_ntff_profile_hook = lambda h: setattr(mod, "_hook", h)
        mod.get_axon_ntff_profile_hook = lambda: mod._hook
        sys.modules["antenv.axon_hooks"] = mod
        from trn_agent_boot.trn_boot import _ntff_profile_via_ctypes

        mod.set_axon_ntff_profile_hook(
            _ntff_profile_via_ctypes("/opt/axon/libaxon_pjrt.so")
        )
    except Exception:
        pass


def _build_graph(use_bias: bool, use_cmask: bool, use_xmask: bool, dbg: bool = False):
    nc = bacc.Bacc("TRN2", target_bir_lowering=False, debug=False, num_devices=N_CORES)

    x_d = nc.dram_tensor("x", [P, NAC * T], BF16, kind="ExternalInput").ap()
    ctx_d = nc.dram_tensor("ctxT", [P, NAC * L], BF16, kind="ExternalInput").ap()
    w_d = {
        name: nc.dram_tensor(name, [P, NAC * 512], BF16, kind="ExternalInput").ap()
        for name in ("wq", "wqr", "wk", "wkr", "wv", "wo")
    }
    tabs_d = nc.dram_tensor("tabs", [P, 2 * T + 2 * L], BF16, kind="ExternalInput").ap()
    if use_bias:
        bias_d = nc.dram_tensor("biases", [1, 6 * 512], BF16, kind="ExternalInput").ap()
    if use_cmask:
        logcm_d = nc.dram_tensor("logcm", [P, NLC], FP32, kind="ExternalInput").ap()
    if use_xmask:
        xmask_d = nc.dram_tensor("xmaskb", [P, T], FP32, kind="ExternalInput").ap()
    out_d = nc.dram_tensor("out", [D_MODEL, T], FP32, kind="ExternalOutput").ap()
    if dbg:
        dbg_d = {
            "qrope": nc.dram_tensor("d_qrope", [P, NAC * T], BF16, kind="ExternalOutput").ap(),
            "krope": nc.dram_tensor("d_krope", [P, NAC * L], BF16, kind="ExternalOutput").ap(),
            "v1": nc.dram_tensor("d_v1", [P, NLC * NUM_HEADS * (HEAD_DIM + 1)], BF16, kind="ExternalOutput").ap(),
            "onorm": nc.dram_tensor("d_onorm", [P, NAC * T], BF16, kind="ExternalOutput").ap(),
            "oraw": nc.dram_tensor("d_oraw", [HEAD_DIM + 1, NUM_HEADS * T], FP32, kind="ExternalOutput").ap(),
            "rec": nc.dram_tensor("d_rec", [1, T], FP32, kind="ExternalOutput").ap(),
            "rb": nc.dram_tensor("d_rb", [HEAD_DIM, T], FP32, kind="ExternalOutput").ap(),
        }

    inv_scale = 1.0 / ATTN_SCALE
    NTG = T // 512  # 2 query column groups of 512
    NLG = L // 512  # 4 key column groups of 512

    with tile.TileContext(nc) as tc:
        with (
            tc.tile_pool(name="const", bufs=1) as const,
            tc.tile_pool(name="big", bufs=1) as big,
            tc.tile_pool(name="tmp", bufs=2) as tmp_pool,
            tc.tile_pool(name="rbp", bufs=2) as rb_pool,
            tc.tile_pool(name="outp", bufs=2) as out_pool,
            tc.tile_pool(name="epool", bufs=4) as e_pool,
        ):
            # ---- input tiles (DMA'd directly as bf16, host layout) ----
            w_bf = {}
            for name in ("wq", "wqr", "wk", "wkr", "wv", "wo"):
                w_bf[name] = big.tile(
                    [P, NAC, 512], BF16, tag=f"w_{name}", name=f"w_{name}"
                )
            x_bf = big.tile([P, NAC, T], BF16)
            ctx_bf = big.tile([P, NAC, L], BF16)
            tabs = const.tile([P, 2 * T + 2 * L], BF16)
            cos_q = tabs[:, 0:T]
            sin_q = tabs[:, T : 2 * T]
            cos_k = tabs[:, 2 * T : 2 * T + L]
            sin_k = tabs[:, 2 * T + L : 2 * T + 2 * L]

            # DMA order = need order: Q-projection inputs first, then K, V, O
            nc.sync.dma_start(x_bf[:], x_d.rearrange("p (c t) -> p c t", c=NAC))
            for name in ("wq", "wqr"):
                nc.sync.dma_start(
                    w_bf[name][:], w_d[name].rearrange("p (c a) -> p c a", c=NAC)
                )
            nc.sync.dma_start(tabs[:, 0 : 2 * T], tabs_d[:, 0 : 2 * T])
            nc.sync.dma_start(ctx_bf[:], ctx_d.rearrange("p (c l) -> p c l", c=NAC))
            for name in ("wk", "wkr"):
                nc.sync.dma_start(
                    w_bf[name][:], w_d[name].rearrange("p (c a) -> p c a", c=NAC)
                )
            nc.sync.dma_start(tabs[:, 2 * T :], tabs_d[:, 2 * T :])
            for name in ("wv", "wo"):
                nc.sync.dma_start(
                    w_bf[name][:], w_d[name].rearrange("p (c a) -> p c a", c=NAC)
                )

            zero_b = const.tile([P, 1], FP32)
            nc.vector.memset(zero_b[:], 0.0)
            if use_cmask:
                logcm_sb = const.tile([P, NLC], FP32)
                nc.sync.dma_start(logcm_sb[:], logcm_d[:])
            if use_xmask:
                xmask_sb = const.tile([P, T], FP32)
                nc.sync.dma_start(xmask_sb[:], xmask_d[:])
            if use_bias:
                bias_bf = const.tile([1, 6, 512], BF16)
                nc.sync.dma_start(
                    bias_bf[:], bias_d.rearrange("p (b a) -> p b a", b=6)
                )
                ones_row = const.tile([1, 512], BF16)
                nc.vector.memset(ones_row[:], 1.0)
                ones_col = const.tile([1, P], BF16)
                nc.vector.memset(ones_col[:], 1.0)

            q_rope = big.tile([P, NAC, T], BF16)
            k_rope = big.tile([P, NAC, L], BF16)
            v1 = big.tile([P, NLC, NUM_HEADS, HEAD_DIM + 1], BF16)
            nc.vector.memset(v1[:, :, :, HEAD_DIM : HEAD_DIM + 1], 1.0)
            o_norm = big.tile([P, NAC, T], BF16)

            with (
                tc.tile_pool(name="psS", bufs=2, space="PSUM") as psS,
                tc.tile_pool(name="psO", bufs=2, space="PSUM") as psO,
            ):
                # ---- projection groups: each uses one [128,1024] s-slot ----
                def rope_combine(out_ap, ps, cos_ap, sin_ap):
                    tt = tmp_pool.tile([P, 512], BF16, tag="ropetmp")
                    nc.vector.tensor_tensor(tt[:], ps[:, 512:1024], sin_ap, op=ALU.mult)
                    nc.vector.tensor_tensor(out_ap, ps[:, 0:512], cos_ap, op=ALU.mult)
                    nc.vector.tensor_tensor(out_ap, out_ap, tt[:], op=ALU.add)

                def q_group(ac, tg):
                    sl = slice(tg * 512, (tg + 1) * 512)
                    ps = psS.tile([P, 1024], FP32, tag="s")
                    for half, wn, bb in ((0, "wq", 0), (1, "wqr", 1)):
                        psl = slice(half * 512, half * 512 + 512)
                        for dc in range(NAC):
                            nc.tensor.matmul(
                                ps[:, psl],
                                lhsT=w_bf[wn][:, dc, ac * P : (ac + 1) * P],
                                rhs=x_bf[:, dc, sl],
                                start=(dc == 0),
                                stop=(dc == NAC - 1) and not use_bias,
                            )
                        if use_bias:
                            nc.tensor.matmul(
                                ps[:, psl],
                                lhsT=bias_bf[:, bb, ac * P : (ac + 1) * P],
                                rhs=ones_row[:],
                                start=False,
                                stop=True,
                            )
                    rope_combine(q_rope[:, ac, sl], ps, cos_q[:, sl], sin_q[:, sl])

                def k_group(ac, g):
                    sl = slice(g * 512, (g + 1) * 512)
                    ps = psS.tile([P, 1024], FP32, tag="s")
                    for half, wn, bb in ((0, "wk", 2), (1, "wkr", 3)):
                        psl = slice(half * 512, half * 512 + 512)
                        for dc in range(NAC):
                            nc.tensor.matmul(
                                ps[:, psl],
                                lhsT=w_bf[wn][:, dc, ac * P : (ac + 1) * P],
                                rhs=ctx_bf[:, dc, sl],
                                start=(dc == 0),
                                stop=(dc == NAC - 1) and not use_bias,
                            )
                        if use_bias:
                            nc.tensor.matmul(
                                ps[:, psl],
                                lhsT=bias_bf[:, bb, ac * P : (ac + 1) * P],
                                rhs=ones_row[:],
                                start=False,
                                stop=True,
                            )
                    rope_combine(k_rope[:, ac, sl], ps, cos_k[:, sl], sin_k[:, sl])

                def v_group(g):
                    # V^T for l-chunks 2g, 2g+1 -> v1[:, lc, h, 0:64]
                    ps = psS.tile([P, 1024], FP32, tag="s")
                    for i in range(2):
                        lc = 2 * g + i
                        psl = slice(i * 512, i * 512 + 512)
                        for dc in range(NAC):
                            nc.tensor.matmul(
                                ps[:, psl],
                                lhsT=ctx_bf[:, dc, lc * P : (lc + 1) * P],
                                rhs=w_bf["wv"][:, dc, :],
                                start=(dc == 0),
                                stop=(dc == NAC - 1) and not use_bias,
                            )
                        if use_bias:
                            nc.tensor.matmul(
                                ps[:, psl],
                                lhsT=ones_col[:],
                                rhs=bias_bf[:, 4, :],
                                start=False,
                                stop=True,
                            )
                        nc.vector.tensor_copy(
                            v1[:, lc, :, 0:HEAD_DIM],
                            ps[:, psl].rearrange("p (h d) -> p h d", d=HEAD_DIM),
                        )

                def out_group(dmc):
                    ps = psS.tile([P, 1024], FP32, tag="s")
                    for tg in range(NTG):
                        sl = slice(tg * 512, (tg + 1) * 512)
                        for ac in range(NAC):
                            nc.tensor.matmul(
                                ps[:, sl],
                                lhsT=w_bf["wo"][:, ac, dmc * P : (dmc + 1) * P],
                                rhs=o_norm[:, ac, sl],
                                start=(ac == 0),
                                stop=(ac == NAC - 1) and not use_bias,
                            )
                        if use_bias:
                            nc.tensor.matmul(
                                ps[:, sl],
                                lhsT=bias_bf[:, 5, dmc * P : (dmc + 1) * P],
                                rhs=ones_row[:],
                                start=False,
                                stop=True,
                            )
                    ot = out_pool.tile([P, T], FP32, tag="ot")
                    if use_xmask:
                        nc.vector.tensor_tensor(ot[:], ps[:], xmask_sb[:], op=ALU.mult)
                    else:
                        nc.vector.tensor_copy(ot[:], ps[:])
                    nc.sync.dma_start(out_d[dmc * P : (dmc + 1) * P, :], ot[:])

                # ---- PE warm-up burst: ~5us of tiny matmuls during the DMA
                # lead-in flips the HAM clock gate to 2.4GHz before real work
                wrm = const.tile([P, 64], BF16)
                nc.vector.memset(wrm[:], 0.0)
                ps_w = psS.tile([P, T], FP32, tag="s")
                for i in range(32):
                    nc.tensor.matmul(
                        ps_w[0:64, 0:64], lhsT=wrm[0:64, :], rhs=wrm[0:64, :],
                        start=True, stop=True,
                    )

                # ---- upfront projections: just enough to start hp0 (first
                # three) plus hp1's prerequisites (absorbed during early hp0)
                q_group(0, 0)
                q_group(0, 1)
                k_group(0, 0)
                q_group(1, 0)
                q_group(1, 1)
                k_group(1, 0)

                # drip-feed schedule: per (hp, q4-iteration) lists of
                # projection groups, sized so every chunk lands >=1 iteration
                # before its first consumer (O emission is deferred 2 iters).
                drip = [
                    [
                        [lambda: k_group(0, 1), lambda: v_group(0)],
                        [lambda: k_group(0, 2), lambda: v_group(1)],
                        [lambda: k_group(0, 3), lambda: v_group(2)],
                        [lambda: v_group(3)],
                        [lambda: v_group(4)],
                        [lambda: v_group(5)],
                        [lambda: v_group(6)],
                        [lambda: v_group(7)],
                    ],
                    [
                        [lambda: k_group(1, 1)],
                        [lambda: k_group(1, 2)],
                        [lambda: k_group(1, 3)],
                        [lambda: q_group(2, 0)],
                        [lambda: q_group(2, 1)],
                        [lambda: k_group(2, 0)],
                        [],
                        [],
                    ],
                    [
                        [lambda: k_group(2, 1)],
                        [lambda: k_group(2, 2)],
                        [lambda: k_group(2, 3)],
                        [lambda: q_group(3, 0)],
                        [lambda: q_group(3, 1)],
                        [lambda: k_group(3, 0)],
                        [],
                        [],
                    ],
                    [
                        [lambda: k_group(3, 1)],
                        [lambda: k_group(3, 2)],
                        [lambda: k_group(3, 3)],
                        [],
                        [],
                        [],
                        [],
                        [],
                    ],
                ]

                # ---- attention, one head pair at a time ----
                for hp in range(NAC):
                    h_a, h_b = 2 * hp, 2 * hp + 1
                    po_a = psO.tile([HEAD_DIM + 1, T], FP32, tag="po")
                    po_b = psO.tile([HEAD_DIM + 1, T], FP32, tag="po")

                    def emit_o(q4, e_a, e_b, po_a=po_a, po_b=po_b, h_a=h_a, h_b=h_b):
                        for tg in range(NTG):
                            sl = slice(tg * 512, (tg + 1) * 512)
                            for lc4 in range(2):
                                lc = q4 * 2 + lc4
                                nc.tensor.matmul(
                                    po_a[:, sl],
                                    lhsT=v1[:, lc, h_a, :],
                                    rhs=e_a[:, lc4, sl],
                                    start=(lc == 0),
                                    stop=(lc == NLC - 1),
                                )
                                nc.tensor.matmul(
                                    po_b[:, sl],
                                    lhsT=v1[:, lc, h_b, :],
                                    rhs=e_b[:, lc4, sl],
                                    start=(lc == 0),
                                    stop=(lc == NLC - 1),
                                )

                    pending = []
                    for q4 in range(8):
                        e_a = e_pool.tile([P, 2, T], BF16, tag="eA")
                        e_b = e_pool.tile([P, 2, T], BF16, tag="eB")
                        for lc4 in range(2):
                            lc = q4 * 2 + lc4
                            s_a = psS.tile([P, T], FP32, tag="s")
                            s_b = psS.tile([P, T], FP32, tag="s")
                            with tc.high_priority():
                                for tg in range(NTG):
                                    sl = slice(tg * 512, (tg + 1) * 512)
                                    nc.tensor.matmul(
                                        s_a[:, sl],
                                        lhsT=k_rope[0:64, hp, lc * P : (lc + 1) * P],
                                        rhs=q_rope[0:64, hp, sl],
                                        start=True,
                                        stop=True,
                                    )
                                    nc.tensor.matmul(
                                        s_b[:, sl],
                                        lhsT=k_rope[64:128, hp, lc * P : (lc + 1) * P],
                                        rhs=q_rope[64:128, hp, sl],
                                        start=True,
                                        stop=True,
                                    )
                            eb = logcm_sb[:, lc : lc + 1] if use_cmask else zero_b[:]
                            nc.scalar.activation(
                                e_a[:, lc4], s_a[:], AF.Exp, bias=eb, scale=inv_scale
                            )
                            nc.scalar.activation(
                                e_b[:, lc4], s_b[:], AF.Exp, bias=eb, scale=inv_scale
                            )
                        for grp in drip[hp][q4]:
                            grp()
                        pending.append((q4, e_a, e_b))
                        if len(pending) > 2:
                            emit_o(*pending.pop(0))
                    while pending:
                        emit_o(*pending.pop(0))

                    # ---- normalization for this head pair ----
                    # Drain both po tiles to SBUF fp32 first (frees the psum
                    # slots immediately). The denominator rows are single-lane
                    # [1,T]; a DVE reciprocal there costs 6.5us, so instead
                    # DMA-scatter both rows across 128 partitions ([128,16]),
                    # invert in ~0.1us, DMA-gather back, broadcast, multiply.
                    o_raws = []
                    for h, po in ((h_a, po_a), (h_b, po_b)):
                        o_raw = rb_pool.tile([HEAD_DIM + 1, T], FP32, tag="oraw")
                        nc.vector.tensor_copy(o_raw[:], po[:])
                        o_raws.append(o_raw)
                        if dbg:
                            nc.sync.dma_start(
                                dbg_d["oraw"][:, h * T : (h + 1) * T], o_raw[:]
                            )
                    dsc = rb_pool.tile([P, 16], FP32, tag="dsc")
                    for i in range(2):
                        nc.sync.dma_start(
                            dsc[:, 8 * i : 8 * i + 8], o_raws[i][64:65, :]
                        )
                    drc = rb_pool.tile([P, 16], FP32, tag="drc")
                    nc.vector.reciprocal(drc[:], dsc[:])
                    for i, h in enumerate((h_a, h_b)):
                        rec = rb_pool.tile([1, T], FP32, tag="rec")
                        nc.sync.dma_start(rec[:], drc[:, 8 * i : 8 * i + 8])
                        rb = rb_pool.tile([HEAD_DIM, T], FP32, tag="rb")
                        nc.gpsimd.partition_broadcast(
                            rb[:], rec[:], channels=HEAD_DIM
                        )
                        nc.vector.tensor_tensor(
                            o_norm[(h % 2) * 64 : (h % 2) * 64 + 64, hp, :],
                            o_raws[i][0:HEAD_DIM, :],
                            rb[:],
                            op=ALU.mult,
                        )
                        if dbg and h == 0:
                            nc.sync.dma_start(dbg_d["rec"][:], rec[:])
                            nc.sync.dma_start(dbg_d["rb"][:], rb[:])

                # ---- output projection + DMA out ----
                for dmc in range(NAC):
                    out_group(dmc)

                if dbg:
                    nc.sync.dma_start(
                        dbg_d["qrope"][:], q_rope[:].rearrange("p a b -> p (a b)")
                    )
                    nc.sync.dma_start(
                        dbg_d["krope"][:], k_rope[:].rearrange("p a b -> p (a b)")
                    )
                    nc.sync.dma_start(
                        dbg_d["v1"][:], v1[:].rearrange("p a b c -> p (a b c)")
                    )
                    nc.sync.dma_start(
                        dbg_d["onorm"][:], o_norm[:].rearrange("p a b -> p (a b)")
                    )

    nc.compile()
    return nc


def _rot_rows(w: np.ndarray) -> np.ndarray:
    """Apply the rotate-half permutation R' on the attn-dim axis (rows):
    row (h,j<32) <- -row (h,32+j);  row (h,32+j) <- +row (h,j)."""
    out = np.empty_like(w)
    for h in range(NUM_HEADS):
        blk = w[h * HEAD_DIM : (h + 1) * HEAD_DIM]
        out[h * HEAD_DIM : h * HEAD_DIM + 32] = -blk[32:64]
        out[h * HEAD_DIM + 32 : (h + 1) * HEAD_DIM] = blk[0:32]
    return out


def _featmajor(w: np.ndarray) -> np.ndarray:
    """[512, N] -> [128, 4, N] bf16 with rows chunked (c*128+p -> [p, c])."""
    n = w.shape[1]
    return np.ascontiguousarray(
        w.reshape(NAC, P, n).transpose(1, 0, 2).reshape(P, NAC * n).astype(BF16NP)
    )


def kernel(
    x,
    context,
    x_mask,
    context_mask,
    Wq_w,
    Wq_b,
    Wk_w,
    Wk_b,
    Wv_w,
    Wv_b,
    Wo_w,
    Wo_b,
    _want_trace=False,
):
    _ensure_ntff_hook()
    x = np.asarray(x, np.float32)
    context = np.asarray(context, np.float32)
    x_mask = np.asarray(x_mask, np.float32)
    context_mask = np.asarray(context_mask, np.float32)
    weights = {
        "wq": _featmajor(np.asarray(Wq_w, np.float32).T),
        "wqr": _featmajor(_rot_rows(np.asarray(Wq_w, np.float32)).T),
        "wk": _featmajor(np.asarray(Wk_w, np.float32).T),
        "wkr": _featmajor(_rot_rows(np.asarray(Wk_w, np.float32)).T),
        "wv": _featmajor(np.asarray(Wv_w, np.float32).T),
        "wo": _featmajor(np.asarray(Wo_w, np.float32).T),
    }
    biases = np.stack(
        [
            np.asarray(Wq_b, np.float32),
            _rot_rows(np.asarray(Wq_b, np.float32)[:, None])[:, 0],
            np.asarray(Wk_b, np.float32),
            _rot_rows(np.asarray(Wk_b, np.float32)[:, None])[:, 0],
            np.asarray(Wv_b, np.float32),
            np.asarray(Wo_b, np.float32),
        ]
    )  # [6, 512]

    use_bias = bool(np.any(biases != 0.0))
    use_cmask = not bool(np.all(context_mask == 1.0))
    use_xmask = not bool(np.all(x_mask == 1.0))

    key = (use_bias, use_cmask, use_xmask)
    if key not in _GRAPH_CACHE:
        _GRAPH_CACHE[key] = _build_graph(*key)
    nc = _GRAPH_CACHE[key]

    len_q = x_mask.sum(axis=(1, 2))  # [B]
    len_k = context_mask.sum(axis=(1, 2))
    theta = (1.0 / (10000.0 ** (np.arange(32, dtype=np.float64) / 32.0))) * ROPE_GAMMA
    theta128 = np.tile(theta, 4)[:, None]  # [128, 1]; row p -> theta_{p%32}

    in_maps = []
    for c in range(N_CORES):
        b, th = c // 2, c % 2
        t0 = th * T
        ang_q = theta128 * ((t0 + np.arange(T)) / len_q[b])[None, :]
        ang_k = theta128 * (np.arange(L) / len_k[b])[None, :]
        tabs = np.concatenate(
            [np.cos(ang_q), np.sin(ang_q), np.cos(ang_k), np.sin(ang_k)], axis=1
        ).astype(BF16NP)
        m = {
            "x": _featmajor(x[b, :, t0 : t0 + T]),
            "ctxT": _featmajor(np.ascontiguousarray(context[b].T)),
            "tabs": np.ascontiguousarray(tabs),
            **weights,
        }
        if use_bias:
            m["biases"] = np.ascontiguousarray(biases.reshape(1, -1).astype(BF16NP))
        if use_cmask:
            with np.errstate(divide="ignore"):
                lcm = np.log(context_mask[b, 0]).astype(np.float32)  # [L]
            m["logcm"] = np.ascontiguousarray(lcm.reshape(NLC, P).T)
        if use_xmask:
            m["xmaskb"] = np.ascontiguousarray(
                np.broadcast_to(x_mask[b, 0, t0 : t0 + T], (P, T))
            )
        in_maps.append(m)

    res = run_bass_kernel_spmd(
        nc, in_maps, core_ids=list(range(N_CORES)), trace=_want_trace
    )
    out = np.empty((B, D_MODEL, T_FULL), np.float32)
    for c in range(N_CORES):
        b, th = c // 2, c % 2
        out[b, :, th * T : (th + 1) * T] = res.results[c]["out"]
    if _want_trace:
        return out, res
    return out
